# revision 33
# baseline (speedup 1.0000x reference)
"""Block-sparse linear layer (x @ (mask*W).T + bias) on 8 TRN2 NeuronCores.

Strategy: data-parallel over batch rows (1024 rows of x per core), with a
mixed-precision K split per output tile. The latin-square block mask keeps 8
of 16 k-blocks per output block-row; of those, the 2 blocks lying in
S = {0, 4, 8, 12} (every 8-long cyclic window contains exactly 2) are
computed in fp8-e4m3 with DoubleRow matmuls (K=256 per instruction, 2x PE
throughput), the other 6 in bf16. That cuts PE time 12.5% while the fp8
quantization noise stays ~1.5e-2 absmax-relative (< 2e-2 tolerance).

All operands are pre-scaled (x by 16, W by 256; exact in bf16, keeps fp8
normal) so every PSUM partial lands in x4096 space; eviction fuses the
1/4096 descale and bias add in one vector/scalar op per half-tile.
"""

import sys
import types

import numpy as np
import ml_dtypes

BATCH = 8192
SIZE = 4096
NB = 16
BLOCK = 256
NCORES = 8
MC = BATCH // NCORES  # 1024 rows per core
P = 128
KS = SIZE // P  # 32 k-subtiles
OT = SIZE // P  # 32 o-tiles
HALF = 512
XS = 16.0
WS = 256.0
FP8_MAX = 240.0

S_BLOCKS = (0, 4, 8, 12)
X8_SLOT_BLOCKS = (0, 4, 8, 12, 0)  # block 0 duplicated so pairs are adjacent
N_WARM = 12

_BUILD_CACHE = {}


def _install_ntff_hook():
    if "antenv.axon_hooks" in sys.modules:
        return
    try:
        from trn_agent_boot.trn_boot import _ntff_profile_via_ctypes

        hook = _ntff_profile_via_ctypes("/opt/axon/libaxon_pjrt.so")
        mod = types.ModuleType("antenv.axon_hooks")
        mod.get_axon_ntff_profile_hook = lambda: hook
        sys.modules["antenv.axon_hooks"] = mod
    except Exception:
        pass


def _keep(i, j):
    return (i + j) % NB >= NB // 2


def _pair_slot(i):
    # fp8 pair of block-row i is (slot k, slot k+1) in X8_SLOT_BLOCKS
    return [2, 1, 0, 3][i // 4]


def _bf16_blocks(i):
    """bf16 blocks of row i, in global first-use order."""
    pair = {X8_SLOT_BLOCKS[_pair_slot(i)], X8_SLOT_BLOCKS[_pair_slot(i) + 1]}
    blocks = [j for j in range(NB) if _keep(i, j) and j not in pair]
    return sorted(blocks, key=_FO.index)


def _first_use_order():
    fo = []
    for i in range(NB):
        pair = {X8_SLOT_BLOCKS[_pair_slot(i)], X8_SLOT_BLOCKS[_pair_slot(i) + 1]}
        for j in range(NB):
            if _keep(i, j) and j not in pair and j not in fo:
                fo.append(j)
    return fo


_FO = _first_use_order()  # 12 bf16 blocks in first-use order
NXB = 2 * len(_FO)  # 24 bf16 x slabs


def _build():
    import concourse.mybir as mybir
    import concourse.tile as tile
    from concourse import bacc

    bf16, f32, f8 = mybir.dt.bfloat16, mybir.dt.float32, mybir.dt.float8e4
    DR = mybir.MatmulPerfMode.DoubleRow
    nc = bacc.Bacc("TRN2", target_bir_lowering=False)

    xb_d = nc.declare_dram_parameter("xb", [P, NXB, MC], bf16, isOutput=False)
    x8_d = nc.declare_dram_parameter("x8", [P, 10, MC], f8, isOutput=False)
    wb_d = nc.declare_dram_parameter("wb", [OT, P, 12, P], bf16, isOutput=False)
    w8_d = nc.declare_dram_parameter("w8", [P, OT * 4, P], f8, isOutput=False)
    bias_d = nc.declare_dram_parameter("biast", [P, OT], f32, isOutput=False)
    out_d = nc.declare_dram_parameter("out", [OT, P, MC], bf16, isOutput=True)

    with tile.TileContext(nc) as tc:
        with (
            tc.tile_pool(name="const", bufs=1) as const_pool,
            tc.tile_pool(name="xbp", bufs=1) as xbp,
            tc.tile_pool(name="x8p", bufs=1) as x8p,
            tc.tile_pool(name="w8p", bufs=1) as w8p,
            tc.tile_pool(name="wbp", bufs=8) as wbp,
            tc.tile_pool(name="opool", bufs=4) as opool,
            tc.tile_pool(name="psum", bufs=4, space="PSUM") as psum_pool,
        ):
            # Warm the PE clock (HAM un-throttles after ~3.4us of sustained
            # gapless matmul activity) while the first DMAs are in flight.
            warm = const_pool.tile([P, HALF], bf16, name="warm")
            nc.gpsimd.memset(warm[:], 0)
            warm_ps = psum_pool.tile([P, HALF], f32, name="warm_ps", tag="ps")
            for i in range(N_WARM):
                nc.tensor.matmul(
                    warm_ps[:],
                    lhsT=warm[:, 0:P],
                    rhs=warm[:],
                    start=(i == 0),
                    stop=(i == N_WARM - 1),
                )

            bias_tile = const_pool.tile([P, OT], f32)
            xb_t = xbp.tile([P, NXB, MC], bf16)
            x8_t = x8p.tile([P, 10, MC], f8)
            w8_t = w8p.tile([P, OT * 4, P], f8)
            wb_tiles = {}

            def wb_dma(t, engine):
                wb_tiles[t] = wbp.tile([P, 12, P], bf16, name="wb")
                engine.dma_start(out=wb_tiles[t][:], in_=wb_d[t])

            def x8_dma(slots, engine, u):
                lo, hi = slots
                engine.dma_start(
                    out=x8_t[:, u * 5 + lo : u * 5 + hi, :],
                    in_=x8_d[:, u * 5 + lo : u * 5 + hi, :],
                )

            def xb_dma(b, engine):
                fi = _FO.index(b)
                engine.dma_start(
                    out=xb_t[:, 2 * fi : 2 * fi + 2, :],
                    in_=xb_d[:, 2 * fi : 2 * fi + 2, :],
                )

            # Startup-critical loads split across BOTH DMA rings so the 16
            # DMA engines pull them concurrently; everything in consumption
            # order (fp8 operands for the first DR matmuls lead, then the
            # first wb tiles and the row-0/1 bf16 x slabs, then bulk).
            x8_dma((2, 4), nc.sync, 0)
            x8_dma((2, 4), nc.gpsimd, 1)
            nc.sync.dma_start(out=w8_t[:, 0:16, :], in_=w8_d[:, 0:16, :])
            nc.gpsimd.dma_start(out=bias_tile[:], in_=bias_d[:])
            wb_dma(0, nc.sync)
            wb_dma(1, nc.gpsimd)

            def xb_dma_u(b, engine, u):
                fi = _FO.index(b)
                engine.dma_start(
                    out=xb_t[:, 2 * fi + u : 2 * fi + u + 1, :],
                    in_=xb_d[:, 2 * fi + u : 2 * fi + u + 1, :],
                )

            # Phase-A-critical x blocks split per-slab across both rings:
            # same descriptor count per ring, half the bytes -> the block's
            # completion semaphore fires ~2x sooner.
            xb_dma_u(9, nc.sync, 0)
            xb_dma_u(9, nc.gpsimd, 1)
            xb_dma_u(10, nc.sync, 0)
            xb_dma_u(10, nc.gpsimd, 1)
            wb_dma(2, nc.sync)
            xb_dma_u(11, nc.sync, 0)
            xb_dma_u(11, nc.gpsimd, 1)
            wb_dma(3, nc.gpsimd)
            xb_dma(13, nc.gpsimd)
            xb_dma(14, nc.sync)
            xb_dma(15, nc.sync)
            xb_dma(7, nc.gpsimd)
            # w8 for rows 2/3 then the remaining bulk streams.
            nc.gpsimd.dma_start(out=w8_t[:, 16:32, :], in_=w8_d[:, 16:32, :])
            x8_dma((1, 2), nc.sync, 0)
            x8_dma((1, 2), nc.sync, 1)
            xb_dma(6, nc.gpsimd)
            wb_dma(4, nc.gpsimd)
            wb_dma(5, nc.gpsimd)
            xb_dma(5, nc.sync)
            nc.gpsimd.dma_start(out=w8_t[:, 32:, :], in_=w8_d[:, 32:, :])
            wb_dma(6, nc.gpsimd)
            wb_dma(7, nc.gpsimd)
            x8_dma((0, 1), nc.sync, 0)
            x8_dma((0, 1), nc.sync, 1)
            xb_dma(3, nc.sync)
            x8_dma((4, 5), nc.sync, 0)
            x8_dma((4, 5), nc.sync, 1)
            xb_dma(2, nc.sync)
            xb_dma(1, nc.sync)

            ps = {}
            n_mm = {}

            def start_tile(t):
                ps[t] = psum_pool.tile([P, MC], f32, name="ps", tag="ps")
                n_mm[t] = [0, 0]

            def mm(t, h, lhsT, rhs, pm=None):
                n_mm[t][h] += 1
                nc.tensor.matmul(
                    ps[t][:, h * HALF : (h + 1) * HALF],
                    lhsT=lhsT,
                    rhs=rhs,
                    start=(n_mm[t][h] == 1),
                    stop=(n_mm[t][h] == 14),
                    perf_mode=pm,
                )

            def dr(t):
                k = _pair_slot(t // 2)
                for u in (0, 1):
                    for h in (0, 1):
                        mm(
                            t,
                            h,
                            w8_t[:, t * 4 + u * 2 : t * 4 + u * 2 + 2, :],
                            x8_t[:, u * 5 + k : u * 5 + k + 2, h * HALF : (h + 1) * HALF],
                            pm=DR,
                        )

            def bf(t, b, u):
                i = t // 2
                sidx = 2 * _bf16_blocks(i).index(b) + u
                fi = _FO.index(b)
                for h in (0, 1):
                    mm(
                        t,
                        h,
                        wb_tiles[t][:, sidx, :],
                        xb_t[:, 2 * fi + u, h * HALF : (h + 1) * HALF],
                    )

            def evict(t):
                # Halves alternate between the Vector and Scalar engines so
                # they run in parallel; the bf16 out tile goes out in one
                # full-tile DMA start (fewer ring descriptors), alternating
                # rings so neither ring's in-order queue backs up.
                o = opool.tile([P, MC], bf16, name="o_tile")
                for q in (0, 1):
                    sl = slice(q * HALF, (q + 1) * HALF)
                    if (t + q) % 2 == 0:
                        nc.vector.tensor_scalar(
                            o[:, sl],
                            ps[t][:, sl],
                            1.0 / (XS * WS),
                            bias_tile[:, t : t + 1],
                            op0=mybir.AluOpType.mult,
                            op1=mybir.AluOpType.add,
                        )
                    else:
                        nc.scalar.activation(
                            o[:, sl],
                            ps[t][:, sl],
                            mybir.ActivationFunctionType.Identity,
                            bias=bias_tile[:, t : t + 1],
                            scale=1.0 / (XS * WS),
                        )
                nc.sync.dma_start(out=out_d[t], in_=o[:])

            def evict_final(t):
                # Separate half-tiles so the two engines run concurrently
                # (same-tile writes serialize), out halves on both rings.
                oa = opool.tile([P, HALF], bf16, name="oa")
                ob = opool.tile([P, HALF], bf16, name="ob")
                nc.vector.tensor_scalar(
                    oa[:],
                    ps[t][:, 0:HALF],
                    1.0 / (XS * WS),
                    bias_tile[:, t : t + 1],
                    op0=mybir.AluOpType.mult,
                    op1=mybir.AluOpType.add,
                )
                nc.scalar.activation(
                    ob[:],
                    ps[t][:, HALF:],
                    mybir.ActivationFunctionType.Identity,
                    bias=bias_tile[:, t : t + 1],
                    scale=1.0 / (XS * WS),
                )
                nc.sync.dma_start(out=out_d[t, :, 0:HALF], in_=oa[:])
                nc.sync.dma_start(out=out_d[t, :, HALF:], in_=ob[:])

            # Rows 0+1 as one 4-tile group: chunk-major over the union of
            # their bf16 blocks maximizes PE work per arriving x slab.
            quad = (0, 1, 2, 3)
            for t in quad:
                start_tile(t)
            for t in quad:
                dr(t)
            union = []
            for b in _bf16_blocks(0) + _bf16_blocks(1):
                if b not in union:
                    union.append(b)
            for b in sorted(union, key=_FO.index):
                for t in quad:
                    if b in _bf16_blocks(t // 2):
                        for u in (0, 1):
                            bf(t, b, u)
            for t in quad:
                evict(t)

            for m in range(2, NB):
                t0, t1 = 2 * m, 2 * m + 1
                if m + 2 < NB:  # prefetch wb two rows ahead
                    wb_dma(t0 + 4, nc.gpsimd)
                    wb_dma(t1 + 4, nc.gpsimd)
                start_tile(t0)
                start_tile(t1)
                last = m == NB - 1
                if last:
                    # tile-major so t30's eviction overlaps t31's matmuls
                    dr(t0)
                    for b in _bf16_blocks(m):
                        for u in (0, 1):
                            bf(t0, b, u)
                    evict_final(t0)
                    dr(t1)
                    for b in _bf16_blocks(m):
                        for u in (0, 1):
                            bf(t1, b, u)
                    evict_final(t1)
                else:
                    dr(t0)
                    dr(t1)
                    for b in _bf16_blocks(m):
                        for t in (t0, t1):
                            for u in (0, 1):
                                bf(t, b, u)
                    evict(t0)
                    evict(t1)
    nc.compile()
    return nc


def _get_kernel():
    if "nc" not in _BUILD_CACHE:
        _BUILD_CACHE["nc"] = _build()
    return _BUILD_CACHE["nc"]


def _expected_mask(mask):
    m4 = np.asarray(mask).reshape(NB, BLOCK, NB, BLOCK)
    keep = m4[:, 0, :, 0]
    if not np.all(m4 == keep[:, None, :, None]):
        return False
    i = np.arange(NB)
    return np.array_equal(keep, ((i[:, None] + i[None, :]) % NB) >= NB // 2)


def _to_fp8(a):
    return np.clip(a, -FP8_MAX, FP8_MAX).astype(ml_dtypes.float8_e4m3)


def kernel(x, weight, bias, mask, _trace=False):
    from concourse.bass_utils import run_bass_kernel_spmd

    _install_ntff_hook()

    x = np.asarray(x)
    weight = np.asarray(weight)
    bias = np.asarray(bias, dtype=np.float32)
    if not _expected_mask(mask):
        w = np.where(np.asarray(mask), weight, 0.0).astype(np.float32)
        out = x.astype(np.float32) @ w.T + bias
        return (out, None) if _trace else out

    nc = _get_kernel()

    ws = (weight * WS).astype(np.float32)  # [out, k]

    # wb[t, p, s, f] = ws[t*P+f, ks(s)*P + p] for the 12 bf16 subtiles of t
    wb = np.empty((OT, P, 12, P), dtype=ml_dtypes.bfloat16)
    # w8[p, t*4 + u*2 + i, f] = ws[t*P+f, blk(k+i)*BLOCK + u*P + p]
    w8 = np.empty((P, OT * 4, P), dtype=ml_dtypes.float8_e4m3)
    for t in range(OT):
        i_row = t // 2
        wt = ws[t * P : (t + 1) * P].reshape(P, KS, P)  # [f, ks, p]
        subs = [2 * b + u for b in _bf16_blocks(i_row) for u in (0, 1)]
        wb[t] = wt[:, subs, :].transpose(2, 1, 0).astype(ml_dtypes.bfloat16)
        k = _pair_slot(i_row)
        for u in (0, 1):
            for i in (0, 1):
                blk = X8_SLOT_BLOCKS[k + i]
                w8[:, t * 4 + u * 2 + i, :] = _to_fp8(
                    wt[:, 2 * blk + u, :].T
                )

    biast = np.ascontiguousarray(bias.reshape(OT, P).T, dtype=np.float32)

    in_maps = []
    for c in range(NCORES):
        xc = x[c * MC : (c + 1) * MC, :].astype(np.float32) * XS  # [MC, SIZE]
        xt = xc.reshape(MC, KS, P).transpose(2, 1, 0)  # [P, KS, MC]
        xb_subs = [2 * b + u for b in _FO for u in (0, 1)]
        xb = np.ascontiguousarray(xt[:, xb_subs, :]).astype(ml_dtypes.bfloat16)
        x8_subs = [2 * b + u for u in (0, 1) for b in X8_SLOT_BLOCKS]
        x8 = _to_fp8(np.ascontiguousarray(xt[:, x8_subs, :]))
        in_maps.append(
            {"xb": xb, "x8": x8, "wb": wb, "w8": w8, "biast": biast}
        )

    res = run_bass_kernel_spmd(nc, in_maps, list(range(NCORES)), trace=_trace)

    out = np.empty((BATCH, SIZE), dtype=np.float32)
    for c in range(NCORES):
        o = res.results[c]["out"]  # [OT, P, MC] bf16
        out[c * MC : (c + 1) * MC, :] = o.reshape(SIZE, MC).T.astype(np.float32)
    if _trace:
        return out, res
    return out


# revision 34
# speedup vs baseline: 1.0037x; 1.0037x over previous
"""Block-sparse linear layer (x @ (mask*W).T + bias) on 8 TRN2 NeuronCores.

Strategy: data-parallel over batch rows (1024 rows of x per core), with a
mixed-precision K split per output tile. The latin-square block mask keeps 8
of 16 k-blocks per output block-row; of those, the 2 blocks lying in
S = {0, 4, 8, 12} (every 8-long cyclic window contains exactly 2) are
computed in fp8-e4m3 with DoubleRow matmuls (K=256 per instruction, 2x PE
throughput), the other 6 in bf16. That cuts PE time 12.5% while the fp8
quantization noise stays ~1.5e-2 absmax-relative (< 2e-2 tolerance).

All operands are pre-scaled (x by 16, W by 256; exact in bf16, keeps fp8
normal) so every PSUM partial lands in x4096 space; eviction fuses the
1/4096 descale and bias add in one vector/scalar op per half-tile.
"""

import sys
import types

import numpy as np
import ml_dtypes

BATCH = 8192
SIZE = 4096
NB = 16
BLOCK = 256
NCORES = 8
MC = BATCH // NCORES  # 1024 rows per core
P = 128
KS = SIZE // P  # 32 k-subtiles
OT = SIZE // P  # 32 o-tiles
HALF = 512
XS = 16.0
WS = 256.0
FP8_MAX = 240.0

S_BLOCKS = (0, 4, 8, 12)
X8_SLOT_BLOCKS = (0, 4, 8, 12, 0)  # block 0 duplicated so pairs are adjacent
N_WARM = 12

_BUILD_CACHE = {}


def _install_ntff_hook():
    if "antenv.axon_hooks" in sys.modules:
        return
    try:
        from trn_agent_boot.trn_boot import _ntff_profile_via_ctypes

        hook = _ntff_profile_via_ctypes("/opt/axon/libaxon_pjrt.so")
        mod = types.ModuleType("antenv.axon_hooks")
        mod.get_axon_ntff_profile_hook = lambda: hook
        sys.modules["antenv.axon_hooks"] = mod
    except Exception:
        pass


def _keep(i, j):
    return (i + j) % NB >= NB // 2


def _pair_slot(i):
    # fp8 pair of block-row i is (slot k, slot k+1) in X8_SLOT_BLOCKS
    return [2, 1, 0, 3][i // 4]


def _bf16_blocks(i):
    """bf16 blocks of row i, in global first-use order."""
    pair = {X8_SLOT_BLOCKS[_pair_slot(i)], X8_SLOT_BLOCKS[_pair_slot(i) + 1]}
    blocks = [j for j in range(NB) if _keep(i, j) and j not in pair]
    return sorted(blocks, key=_FO.index)


def _first_use_order():
    fo = []
    for i in range(NB):
        pair = {X8_SLOT_BLOCKS[_pair_slot(i)], X8_SLOT_BLOCKS[_pair_slot(i) + 1]}
        for j in range(NB):
            if _keep(i, j) and j not in pair and j not in fo:
                fo.append(j)
    return fo


_FO = _first_use_order()  # 12 bf16 blocks in first-use order
NXB = 2 * len(_FO)  # 24 bf16 x slabs


def _build():
    import concourse.mybir as mybir
    import concourse.tile as tile
    from concourse import bacc

    bf16, f32, f8 = mybir.dt.bfloat16, mybir.dt.float32, mybir.dt.float8e4
    DR = mybir.MatmulPerfMode.DoubleRow
    nc = bacc.Bacc("TRN2", target_bir_lowering=False)

    xb_d = nc.declare_dram_parameter("xb", [P, NXB, MC], bf16, isOutput=False)
    x8_d = nc.declare_dram_parameter("x8", [P, 10, MC], f8, isOutput=False)
    wb_d = nc.declare_dram_parameter("wb", [OT, P, 12, P], bf16, isOutput=False)
    w8_d = nc.declare_dram_parameter("w8", [P, OT * 4, P], f8, isOutput=False)
    bias_d = nc.declare_dram_parameter("biast", [P, OT], f32, isOutput=False)
    out_d = nc.declare_dram_parameter("out", [OT, P, MC], bf16, isOutput=True)

    with tile.TileContext(nc) as tc:
        with (
            tc.tile_pool(name="const", bufs=1) as const_pool,
            tc.tile_pool(name="xbp", bufs=1) as xbp,
            tc.tile_pool(name="x8p", bufs=1) as x8p,
            tc.tile_pool(name="w8p", bufs=1) as w8p,
            tc.tile_pool(name="wbp", bufs=8) as wbp,
            tc.tile_pool(name="opool", bufs=4) as opool,
            tc.tile_pool(name="psum", bufs=4, space="PSUM") as psum_pool,
        ):
            # Warm the PE clock (HAM un-throttles after ~3.4us of sustained
            # gapless matmul activity) while the first DMAs are in flight.
            warm = const_pool.tile([P, HALF], bf16, name="warm")
            nc.gpsimd.memset(warm[:], 0)
            warm_ps = psum_pool.tile([P, HALF], f32, name="warm_ps", tag="ps")
            for i in range(N_WARM):
                nc.tensor.matmul(
                    warm_ps[:],
                    lhsT=warm[:, 0:P],
                    rhs=warm[:],
                    start=(i == 0),
                    stop=(i == N_WARM - 1),
                )

            bias_tile = const_pool.tile([P, OT], f32)
            xb_t = xbp.tile([P, NXB, MC], bf16)
            x8_t = x8p.tile([P, 10, MC], f8)
            w8_t = w8p.tile([P, OT * 4, P], f8)
            wb_tiles = {}

            def wb_dma(t, engine):
                wb_tiles[t] = wbp.tile([P, 12, P], bf16, name="wb")
                engine.dma_start(out=wb_tiles[t][:], in_=wb_d[t])

            def x8_dma(slots, engine, u):
                lo, hi = slots
                engine.dma_start(
                    out=x8_t[:, u * 5 + lo : u * 5 + hi, :],
                    in_=x8_d[:, u * 5 + lo : u * 5 + hi, :],
                )

            def xb_dma(b, engine):
                fi = _FO.index(b)
                engine.dma_start(
                    out=xb_t[:, 2 * fi : 2 * fi + 2, :],
                    in_=xb_d[:, 2 * fi : 2 * fi + 2, :],
                )

            # Startup-critical loads split across BOTH DMA rings so the 16
            # DMA engines pull them concurrently; everything in consumption
            # order (fp8 operands for the first DR matmuls lead, then the
            # first wb tiles and the row-0/1 bf16 x slabs, then bulk).
            x8_dma((2, 4), nc.sync, 0)
            x8_dma((2, 4), nc.gpsimd, 1)
            nc.sync.dma_start(out=w8_t[:, 0:16, :], in_=w8_d[:, 0:16, :])
            nc.gpsimd.dma_start(out=bias_tile[:], in_=bias_d[:])
            wb_dma(0, nc.sync)
            wb_dma(1, nc.gpsimd)
            xb_dma(9, nc.sync)
            xb_dma(10, nc.gpsimd)
            wb_dma(2, nc.gpsimd)
            xb_dma(11, nc.sync)
            wb_dma(3, nc.gpsimd)
            xb_dma(13, nc.gpsimd)
            xb_dma(14, nc.sync)
            xb_dma(15, nc.sync)
            xb_dma(7, nc.gpsimd)
            # w8 for rows 2/3 then the remaining bulk streams.
            nc.gpsimd.dma_start(out=w8_t[:, 16:32, :], in_=w8_d[:, 16:32, :])
            x8_dma((1, 2), nc.sync, 0)
            x8_dma((1, 2), nc.sync, 1)
            xb_dma(6, nc.gpsimd)
            wb_dma(4, nc.gpsimd)
            wb_dma(5, nc.gpsimd)
            xb_dma(5, nc.sync)
            nc.gpsimd.dma_start(out=w8_t[:, 32:, :], in_=w8_d[:, 32:, :])
            wb_dma(6, nc.gpsimd)
            wb_dma(7, nc.gpsimd)
            x8_dma((0, 1), nc.sync, 0)
            x8_dma((0, 1), nc.sync, 1)
            xb_dma(3, nc.sync)
            x8_dma((4, 5), nc.sync, 0)
            x8_dma((4, 5), nc.sync, 1)
            xb_dma(2, nc.sync)
            xb_dma(1, nc.sync)

            ps = {}
            n_mm = {}

            def start_tile(t):
                ps[t] = psum_pool.tile([P, MC], f32, name="ps", tag="ps")
                n_mm[t] = [0, 0]

            def mm(t, h, lhsT, rhs, pm=None):
                n_mm[t][h] += 1
                nc.tensor.matmul(
                    ps[t][:, h * HALF : (h + 1) * HALF],
                    lhsT=lhsT,
                    rhs=rhs,
                    start=(n_mm[t][h] == 1),
                    stop=(n_mm[t][h] == 14),
                    perf_mode=pm,
                )

            def dr(t):
                k = _pair_slot(t // 2)
                for u in (0, 1):
                    for h in (0, 1):
                        mm(
                            t,
                            h,
                            w8_t[:, t * 4 + u * 2 : t * 4 + u * 2 + 2, :],
                            x8_t[:, u * 5 + k : u * 5 + k + 2, h * HALF : (h + 1) * HALF],
                            pm=DR,
                        )

            def bf(t, b, u):
                i = t // 2
                sidx = 2 * _bf16_blocks(i).index(b) + u
                fi = _FO.index(b)
                for h in (0, 1):
                    mm(
                        t,
                        h,
                        wb_tiles[t][:, sidx, :],
                        xb_t[:, 2 * fi + u, h * HALF : (h + 1) * HALF],
                    )

            def evict(t):
                # Halves alternate between the Vector and Scalar engines so
                # they run in parallel; the bf16 out tile goes out in one
                # full-tile DMA start (fewer ring descriptors), alternating
                # rings so neither ring's in-order queue backs up.
                o = opool.tile([P, MC], bf16, name="o_tile")
                for q in (0, 1):
                    sl = slice(q * HALF, (q + 1) * HALF)
                    if (t + q) % 2 == 0:
                        nc.vector.tensor_scalar(
                            o[:, sl],
                            ps[t][:, sl],
                            1.0 / (XS * WS),
                            bias_tile[:, t : t + 1],
                            op0=mybir.AluOpType.mult,
                            op1=mybir.AluOpType.add,
                        )
                    else:
                        nc.scalar.activation(
                            o[:, sl],
                            ps[t][:, sl],
                            mybir.ActivationFunctionType.Identity,
                            bias=bias_tile[:, t : t + 1],
                            scale=1.0 / (XS * WS),
                        )
                nc.sync.dma_start(out=out_d[t], in_=o[:])

            def evict_final(t):
                # Separate half-tiles so the two engines run concurrently
                # (same-tile writes serialize), out halves on both rings.
                oa = opool.tile([P, HALF], bf16, name="oa")
                ob = opool.tile([P, HALF], bf16, name="ob")
                nc.vector.tensor_scalar(
                    oa[:],
                    ps[t][:, 0:HALF],
                    1.0 / (XS * WS),
                    bias_tile[:, t : t + 1],
                    op0=mybir.AluOpType.mult,
                    op1=mybir.AluOpType.add,
                )
                nc.scalar.activation(
                    ob[:],
                    ps[t][:, HALF:],
                    mybir.ActivationFunctionType.Identity,
                    bias=bias_tile[:, t : t + 1],
                    scale=1.0 / (XS * WS),
                )
                nc.sync.dma_start(out=out_d[t, :, 0:HALF], in_=oa[:])
                nc.sync.dma_start(out=out_d[t, :, HALF:], in_=ob[:])

            # Rows 0+1 as one 4-tile group: chunk-major over the union of
            # their bf16 blocks maximizes PE work per arriving x slab.
            quad = (0, 1, 2, 3)
            for t in quad:
                start_tile(t)
            for t in quad:
                dr(t)
            union = []
            for b in _bf16_blocks(0) + _bf16_blocks(1):
                if b not in union:
                    union.append(b)
            for b in sorted(union, key=_FO.index):
                for t in quad:
                    if b in _bf16_blocks(t // 2):
                        for u in (0, 1):
                            bf(t, b, u)
            for t in quad:
                evict(t)

            for m in range(2, NB):
                t0, t1 = 2 * m, 2 * m + 1
                if m + 2 < NB:  # prefetch wb two rows ahead
                    wb_dma(t0 + 4, nc.gpsimd)
                    wb_dma(t1 + 4, nc.gpsimd)
                start_tile(t0)
                start_tile(t1)
                last = m == NB - 1
                if last:
                    # tile-major so t30's eviction overlaps t31's matmuls
                    dr(t0)
                    for b in _bf16_blocks(m):
                        for u in (0, 1):
                            bf(t0, b, u)
                    evict_final(t0)
                    dr(t1)
                    for b in _bf16_blocks(m):
                        for u in (0, 1):
                            bf(t1, b, u)
                    evict_final(t1)
                else:
                    dr(t0)
                    dr(t1)
                    for b in _bf16_blocks(m):
                        for t in (t0, t1):
                            for u in (0, 1):
                                bf(t, b, u)
                    evict(t0)
                    evict(t1)
    nc.compile()
    return nc


def _get_kernel():
    if "nc" not in _BUILD_CACHE:
        _BUILD_CACHE["nc"] = _build()
    return _BUILD_CACHE["nc"]


def _expected_mask(mask):
    m4 = np.asarray(mask).reshape(NB, BLOCK, NB, BLOCK)
    keep = m4[:, 0, :, 0]
    if not np.all(m4 == keep[:, None, :, None]):
        return False
    i = np.arange(NB)
    return np.array_equal(keep, ((i[:, None] + i[None, :]) % NB) >= NB // 2)


def _to_fp8(a):
    return np.clip(a, -FP8_MAX, FP8_MAX).astype(ml_dtypes.float8_e4m3)


def kernel(x, weight, bias, mask, _trace=False):
    from concourse.bass_utils import run_bass_kernel_spmd

    _install_ntff_hook()

    x = np.asarray(x)
    weight = np.asarray(weight)
    bias = np.asarray(bias, dtype=np.float32)
    if not _expected_mask(mask):
        w = np.where(np.asarray(mask), weight, 0.0).astype(np.float32)
        out = x.astype(np.float32) @ w.T + bias
        return (out, None) if _trace else out

    nc = _get_kernel()

    ws = (weight * WS).astype(np.float32)  # [out, k]

    # wb[t, p, s, f] = ws[t*P+f, ks(s)*P + p] for the 12 bf16 subtiles of t
    wb = np.empty((OT, P, 12, P), dtype=ml_dtypes.bfloat16)
    # w8[p, t*4 + u*2 + i, f] = ws[t*P+f, blk(k+i)*BLOCK + u*P + p]
    w8 = np.empty((P, OT * 4, P), dtype=ml_dtypes.float8_e4m3)
    for t in range(OT):
        i_row = t // 2
        wt = ws[t * P : (t + 1) * P].reshape(P, KS, P)  # [f, ks, p]
        subs = [2 * b + u for b in _bf16_blocks(i_row) for u in (0, 1)]
        wb[t] = wt[:, subs, :].transpose(2, 1, 0).astype(ml_dtypes.bfloat16)
        k = _pair_slot(i_row)
        for u in (0, 1):
            for i in (0, 1):
                blk = X8_SLOT_BLOCKS[k + i]
                w8[:, t * 4 + u * 2 + i, :] = _to_fp8(
                    wt[:, 2 * blk + u, :].T
                )

    biast = np.ascontiguousarray(bias.reshape(OT, P).T, dtype=np.float32)

    in_maps = []
    for c in range(NCORES):
        xc = x[c * MC : (c + 1) * MC, :].astype(np.float32) * XS  # [MC, SIZE]
        xt = xc.reshape(MC, KS, P).transpose(2, 1, 0)  # [P, KS, MC]
        xb_subs = [2 * b + u for b in _FO for u in (0, 1)]
        xb = np.ascontiguousarray(xt[:, xb_subs, :]).astype(ml_dtypes.bfloat16)
        x8_subs = [2 * b + u for u in (0, 1) for b in X8_SLOT_BLOCKS]
        x8 = _to_fp8(np.ascontiguousarray(xt[:, x8_subs, :]))
        in_maps.append(
            {"xb": xb, "x8": x8, "wb": wb, "w8": w8, "biast": biast}
        )

    res = run_bass_kernel_spmd(nc, in_maps, list(range(NCORES)), trace=_trace)

    out = np.empty((BATCH, SIZE), dtype=np.float32)
    for c in range(NCORES):
        o = res.results[c]["out"]  # [OT, P, MC] bf16
        out[c * MC : (c + 1) * MC, :] = o.reshape(SIZE, MC).T.astype(np.float32)
    if _trace:
        return out, res
    return out
